# revision 27
# baseline (speedup 1.0000x reference)
"""Bot-detection transformer forward pass on 8 Trainium2 NeuronCores.

Strategy: data-parallel over batch (B=8 -> 1 sequence per core, no
collectives).  Residual stream kept transposed (feature-major, xT:
[768 x 1024] f32 as 6 partition-tiles).

Mixed precision chosen by measured error contribution: the attention
path (qkv projections, scores, softmax weights, att*V) runs in fp8e4
DoubleRow matmuls (K=256/instr, 2x bf16 throughput) -- softmax's
weighted averaging over ~500 tokens washes out fp8 quantization noise.
The per-token direct paths (attention out-projection, feed-forward)
stay bf16: fp8 noise there flows straight to the logits.  fp8 weights
carry power-of-two scales compensated for free inside the softmax exp
(activation scale) and the v mask-fold.

LayerNorm scale/bias are folded into downstream projection weights
host-side; on-device LN is (x - mean) * rstd with the row broadcasts
on the otherwise-idle GPSIMD engine instead of PE matmuls.  Softmax
needs no max subtraction (scores bounded); the key-padding mask folds
into v; the softmax denominator is a 65th stationary column of v.
"""

import math

import numpy as np

B, S, D, H, L, V, C = 8, 1024, 768, 12, 6, 32000, 2
HD, DF, MAXPOS = 64, 3072, 2048
P = 128
KT = D // P    # 6 feature tiles
NT = S // P    # 8 token tiles
FT = DF // P   # 24 ff tiles
NQ = 2         # query halves of 512
QW = S // NQ   # 512
EPS = 1e-5
N_CORES = 8

SQK = 64.0     # fp8 scale for q and k projections (each also folds 1/sqrt(8))
SV = 64.0      # fp8 scale for v projection (inverse folded into mask column)

_CACHE = {}


def _build_nc(n_layers=L):
    import os
    import concourse.bass as bass
    import concourse.tile as tile
    from concourse import bacc, mybir
    from concourse.bass import ds, ts
    from concourse.masks import make_identity
    from contextlib import ExitStack

    f32 = mybir.dt.float32
    bf16 = mybir.dt.bfloat16
    f32r = mybir.dt.float32r
    fp8 = mybir.dt.float8e4
    i32 = mybir.dt.int32
    AF = mybir.ActivationFunctionType
    OP = mybir.AluOpType
    DR = mybir.MatmulPerfMode.DoubleRow

    nc = bacc.Bacc("TRN2", target_bir_lowering=False, debug=False)

    # ---------------- DRAM I/O ----------------
    d_ids = nc.dram_tensor("ids", [P, NT], i32, kind="ExternalInput")
    d_gm = nc.dram_tensor("gmask", [P, NT], f32, kind="ExternalInput")
    d_emb = nc.dram_tensor("emb", [V, D], f32, kind="ExternalInput")
    d_posT = nc.dram_tensor("posT", [D, S], bf16, kind="ExternalInput")
    d_qkw = nc.dram_tensor("qkw", [L, 12, D, P], fp8, kind="ExternalInput")
    d_qkb = nc.dram_tensor("qkb", [L, P, 12], f32, kind="ExternalInput")
    d_vw = nc.dram_tensor("vw", [L, KT, P, D], fp8, kind="ExternalInput")
    d_vb = nc.dram_tensor("vb", [L, D], fp8, kind="ExternalInput")
    d_ow = nc.dram_tensor("ow", [L, KT, D, P], bf16, kind="ExternalInput")
    d_ob = nc.dram_tensor("ob", [L, P, KT], f32, kind="ExternalInput")
    d_f1w = nc.dram_tensor("f1w", [L, FT, D, P], bf16, kind="ExternalInput")
    d_f1b = nc.dram_tensor("f1b", [L, P, FT], f32, kind="ExternalInput")
    d_f2w = nc.dram_tensor("f2w", [L, KT, DF, P], bf16, kind="ExternalInput")
    d_f2b = nc.dram_tensor("f2b", [L, P, KT], f32, kind="ExternalInput")
    d_hls = nc.dram_tensor("hls", [P, KT], f32, kind="ExternalInput")
    d_hlb = nc.dram_tensor("hlb", [P, KT], f32, kind="ExternalInput")
    d_cw = nc.dram_tensor("cw", [D, C], f32, kind="ExternalInput")
    d_cb = nc.dram_tensor("cb", [1, C], f32, kind="ExternalInput")
    d_out = nc.dram_tensor("out", [1, C], f32, kind="ExternalOutput")

    with tile.TileContext(nc) as tc, ExitStack() as ctx:
        # ---------------- pools ----------------
        state = ctx.enter_context(tc.tile_pool(name="state", bufs=1))
        consts = ctx.enter_context(tc.tile_pool(name="consts", bufs=1))
        hpool = ctx.enter_context(tc.tile_pool(name="hpool", bufs=1))
        h2pool = ctx.enter_context(tc.tile_pool(name="h2pool", bufs=1))
        qkpool = ctx.enter_context(tc.tile_pool(name="qkpool", bufs=1))
        fpool = ctx.enter_context(tc.tile_pool(name="fpool", bufs=1))
        vpool = ctx.enter_context(tc.tile_pool(name="vpool", bufs=1))
        attp = ctx.enter_context(tc.tile_pool(name="attp", bufs=1))
        vwpool = ctx.enter_context(tc.tile_pool(name="vwpool", bufs=1))
        w8p = ctx.enter_context(tc.tile_pool(name="w8p", bufs=2))
        wbf = ctx.enter_context(tc.tile_pool(name="wbf", bufs=4))
        wff2 = ctx.enter_context(tc.tile_pool(name="wff2", bufs=3))
        epool = ctx.enter_context(tc.tile_pool(name="epool", bufs=2))
        tmp = ctx.enter_context(tc.tile_pool(name="tmp", bufs=2))
        embp = ctx.enter_context(tc.tile_pool(name="embp", bufs=1))
        zpool = ctx.enter_context(tc.tile_pool(name="zpool", bufs=1))
        rows = ctx.enter_context(tc.tile_pool(name="rows", bufs=2))
        srows = ctx.enter_context(tc.tile_pool(name="srows", bufs=2))
        bcast = ctx.enter_context(tc.tile_pool(name="bcast", bufs=2))
        params = ctx.enter_context(tc.tile_pool(name="params", bufs=2))
        # PSUM budget: 8 banks -- dedicated pools per producer class so
        # tile-ring FIFO reuse cannot serialize across pipeline phases.
        # pscore: score pairs (2 banks x 2); pqkv: qkv-proj accs + LN
        # stats + embedding (1 bank x 2); pmm: ffn-chain accs (1 x 1);
        # patt: att*V accumulators (1 x 1).
        _cfg = os.environ.get("KCFG", "2,1,1,2").split(",")
        _ps_b, _pmm_b, _patt_b, _pq_b = (int(x) for x in _cfg)
        pscore = ctx.enter_context(
            tc.tile_pool(name="pscore", bufs=_ps_b, space="PSUM"))
        pmm = ctx.enter_context(tc.tile_pool(name="pmm", bufs=_pmm_b, space="PSUM"))
        patt = ctx.enter_context(
            tc.tile_pool(name="patt", bufs=_patt_b, space="PSUM"))
        pqkv = (
            ctx.enter_context(tc.tile_pool(name="pqkv", bufs=_pq_b, space="PSUM"))
            if _pq_b
            else pmm
        )
        _pq_tag = "pq" if _pq_b else "pmm"

        # ---------------- constants ----------------
        xT = state.tile([P, KT, S], f32r, tag="xT")
        ones_f32 = consts.tile([P, 1], f32, tag="ones_f32")
        nc.vector.memset(ones_f32[:, :], 1.0)
        ones_col = consts.tile([P, 1], f32r, tag="ones_col")
        nc.vector.tensor_copy(out=ones_col[:, :], in_=ones_f32[:, :])
        ones_rf32 = consts.tile([1, QW], f32, tag="ones_rf32")
        nc.vector.memset(ones_rf32[:, :], 1.0)
        ones_row8 = consts.tile([1, P], fp8, tag="ones_row8")
        nc.vector.tensor_copy(out=ones_row8[:, :], in_=ones_rf32[:, 0:P])
        ones_rowr = consts.tile([1, QW], f32r, tag="ones_rowr")
        nc.vector.tensor_copy(out=ones_rowr[:, :], in_=ones_rf32[:, :])
        ident = consts.tile([P, P], f32, tag="ident")
        make_identity(nc, ident[:, :])
        eps_sb = consts.tile([1, 1], f32, tag="eps")
        nc.vector.memset(eps_sb[:, :], EPS)
        ids_sb = consts.tile([P, NT], i32, tag="ids")
        nc.sync.dma_start(out=ids_sb[:, :], in_=d_ids[:, :])
        gcol = consts.tile([P, NT], f32, tag="gcol")
        nc.sync.dma_start(out=gcol[:, :], in_=d_gm[:, :])
        # mask / SV : folds v's fp8 weight scale away at the mask multiply
        gcolv = consts.tile([P, NT], f32, tag="gcolv")
        nc.vector.tensor_scalar(
            out=gcolv[:, :], in0=gcol[:, :], scalar1=1.0 / SV, scalar2=None,
            op0=OP.mult,
        )
        gcol8 = consts.tile([P, NT], fp8, tag="gcol8")
        nc.vector.tensor_copy(out=gcol8[:, :], in_=gcol[:, :])
        hls_sb = consts.tile([P, KT], f32, tag="hls")
        nc.sync.dma_start(out=hls_sb[:, :], in_=d_hls[:, :])
        hlb_sb = consts.tile([P, KT], f32, tag="hlb")
        nc.sync.dma_start(out=hlb_sb[:, :], in_=d_hlb[:, :])
        cw_sb = consts.tile([P, KT, C], f32r, tag="cw")
        nc.sync.dma_start(
            out=cw_sb[:, :, :],
            in_=d_cw.rearrange("(j p) c -> p j c", p=P).bitcast(f32r),
        )
        cb_sb = consts.tile([1, C], f32r, tag="cb")
        nc.sync.dma_start(out=cb_sb[:, :], in_=d_cb[:, :].bitcast(f32r))

        # ---------------- embedding ----------------
        posT_sb = attp.tile([P, KT, S], bf16, tag="attT")
        nc.sync.dma_start(
            out=posT_sb[:, :, :], in_=d_posT.rearrange("(j p) s -> p j s", p=P)
        )
        for t in range(NT):
            embt = embp.tile([P, D], f32, tag="embt")
            nc.gpsimd.indirect_dma_start(
                out=embt[:, :],
                out_offset=None,
                in_=d_emb[:, :],
                in_offset=bass.IndirectOffsetOnAxis(ap=ids_sb[:, t : t + 1], axis=0),
            )
            ptr0 = pqkv.tile([P, QW], f32, tag=_pq_tag)
            for j in range(4):
                nc.tensor.transpose(
                    out=ptr0[:, j * P : (j + 1) * P],
                    in_=embt[:, j * P : (j + 1) * P],
                    identity=ident[:, :],
                )
            ptr1 = pqkv.tile([P, QW], f32, tag=_pq_tag)
            for j in range(2):
                nc.tensor.transpose(
                    out=ptr1[:, j * P : (j + 1) * P],
                    in_=embt[:, (4 + j) * P : (5 + j) * P],
                    identity=ident[:, :],
                )
            nc.vector.tensor_tensor(
                out=xT[:, 0:4, t * P : (t + 1) * P],
                in0=ptr0[:, :].rearrange("p (a b) -> p a b", b=P),
                in1=posT_sb[:, 0:4, t * P : (t + 1) * P],
                op=OP.add,
            )
            nc.vector.tensor_tensor(
                out=xT[:, 4:6, t * P : (t + 1) * P],
                in0=ptr1[:, 0 : 2 * P].rearrange("p (a b) -> p a b", b=P),
                in1=posT_sb[:, 4:6, t * P : (t + 1) * P],
                op=OP.add,
            )

        # ---------------- layer norm helper (one query half) ----------------
        def layer_norm_half(src, dst, n):
            """src (P,KT,S) f32r -> dst[:, :, half n] = (src - mean)*rstd
            over the feature axis.  gamma/beta folded into downstream
            weights host-side.  Elementwise work alternates DVE/Pool."""
            nsl = ds(n * QW, QW)
            pstat = pqkv.tile([P, QW], f32, tag=_pq_tag)
            pstq = pqkv.tile([P, QW], f32, tag=_pq_tag)
            psum, psq = pstat[0:1, :], pstq[0:1, :]
            for j in range(KT):
                xsq = tmp.tile([P, QW], f32r, tag="tmp")
                eng = nc.gpsimd if j % 2 == 0 else nc.vector
                eng.tensor_tensor(
                    out=xsq[:, :], in0=src[:, j, nsl], in1=src[:, j, nsl],
                    op=OP.mult,
                )
                nc.tensor.matmul(
                    psum, ones_col[:, :], src[:, j, nsl],
                    start=(j == 0), stop=(j == KT - 1),
                )
                nc.tensor.matmul(
                    psq, ones_col[:, :], xsq[:, :],
                    start=(j == 0), stop=(j == KT - 1),
                )
            mean = srows.tile([1, QW], bf16, tag="mean")
            nc.vector.tensor_scalar(
                out=mean[:, :], in0=psum, scalar1=1.0 / D, scalar2=None,
                op0=OP.mult,
            )
            msq = srows.tile([1, QW], bf16, tag="srow")
            nc.vector.tensor_scalar(
                out=msq[:, :], in0=psq, scalar1=1.0 / D, scalar2=None,
                op0=OP.mult,
            )
            var = srows.tile([1, QW], bf16, tag="srow")
            nc.vector.scalar_tensor_tensor(
                out=var[:, :], in0=mean[:, :], scalar=-1.0, in1=mean[:, :],
                op0=OP.mult, op1=OP.mult,
            )
            nc.vector.tensor_tensor(
                out=var[:, :], in0=var[:, :], in1=msq[:, :], op=OP.add,
            )
            lnv = srows.tile([1, QW], bf16, tag="srow")
            nc.scalar.activation(lnv[:, :], var[:, :], AF.Ln, bias=eps_sb[:, :])
            rstd = srows.tile([1, QW], bf16, tag="rstd")
            nc.scalar.activation(rstd[:, :], lnv[:, :], AF.Exp, scale=-0.5)
            mb = bcast.tile([P, QW], bf16, tag="mrb")
            nc.gpsimd.partition_broadcast(mb[:, :], mean[:, :], channels=P)
            rb = bcast.tile([P, QW], bf16, tag="mrb")
            nc.gpsimd.partition_broadcast(rb[:, :], rstd[:, :], channels=P)
            for j in range(KT):
                xc = tmp.tile([P, QW], f32, tag="tmp")
                nc.gpsimd.tensor_tensor(
                    out=xc[:, :], in0=src[:, j, nsl], in1=mb[:, :],
                    op=OP.subtract,
                )
                nc.vector.tensor_tensor(
                    out=dst[:, j, nsl], in0=xc[:, :], in1=rb[:, :],
                    op=OP.mult,
                )

        # ---------------- layers ----------------
        for l in range(n_layers):
            # ---- LN1 + q,k,v projections, emitted per query half so the
            # half-0 work (and its score/exp stream) can run in the
            # shadow of the previous layer's half-1 ffn chain ----
            # qk_sb layout: [P, 6, 2, S]; g in 0..2 holds q for heads
            # 4g..4g+3 (partition 32*slot + d%32, free axis 2 = d//32),
            # g+3 same for k.  DoubleRow score matmuls contract the 64
            # head dims as 32 partitions x 2 subtiles.
            hT = hpool.tile([P, KT, S], fp8, tag="h")
            qkb_sb = params.tile([P, 12], f32, tag="qkb")
            nc.sync.dma_start(out=qkb_sb[:, :], in_=d_qkb[l])
            qk_sb = qkpool.tile([P, 6, 2, S], fp8, tag="qk")
            vw_sb = vwpool.tile([P, KT, D], fp8, tag="vw")
            nc.sync.dma_start(
                out=vw_sb[:, :, :], in_=d_vw[l].rearrange("j p d -> p j d"),
            )
            vb_row = rows.tile([1, D], fp8, tag="brow")
            nc.sync.dma_start(out=vb_row[:, :], in_=d_vb[l : l + 1, :])
            v_sb = vpool.tile([P, H, NT, 80], fp8, tag="v")
            nc.vector.memset(v_sb[:, :, :, 65:66], 0.0)
            for n in range(NQ):
                nsl = ds(n * QW, QW)
                layer_norm_half(xT, hT, n)
                for tg in range(3):
                    wqk4 = w8p.tile([P, 4, KT, P], fp8, tag="w8")
                    nc.sync.dma_start(
                        out=wqk4[:, :, :, :],
                        in_=d_qkw[l, 4 * tg : 4 * tg + 4]
                        .rearrange("t (j p) c -> p t j c", p=P),
                    )
                    for tt in range(4):
                        t = 4 * tg + tt
                        wt = wqk4[:, tt]
                        gq, sub = (
                            (t // 2, t % 2) if t < 6 else (3 + (t - 6) // 2, t % 2)
                        )
                        acc = pqkv.tile([P, QW], f32, tag=_pq_tag)
                        for j in range(KT // 2):
                            nc.tensor.matmul(
                                acc[:, :],
                                wt[:, 2 * j : 2 * j + 2, :],
                                hT[:, 2 * j : 2 * j + 2, nsl],
                                start=(j == 0), stop=(j == KT // 2 - 1),
                                perf_mode=DR,
                            )
                        nc.vector.tensor_scalar(
                            out=qk_sb[:, gq, sub, nsl], in0=acc[:, :],
                            scalar1=qkb_sb[:, t : t + 1], scalar2=None, op0=OP.add,
                        )
                for t in range(4 * n, 4 * n + 4):
                    for c0, cn in ((0, QW), (QW, D - QW)):
                        acc = pqkv.tile([P, QW], f32, tag=_pq_tag)
                        nc.tensor.matmul(
                            acc[:, 0:cn], ones_row8[0:1, 0:P],
                            vb_row[:, c0 : c0 + cn],
                            start=True, stop=False,
                        )
                        for j in range(KT // 2):
                            nc.tensor.matmul(
                                acc[:, 0:cn],
                                hT[:, 2 * j : 2 * j + 2, ts(t, P)],
                                vw_sb[:, 2 * j : 2 * j + 2, c0 : c0 + cn],
                                start=False, stop=(j == KT // 2 - 1),
                                perf_mode=DR,
                            )
                        nc.vector.tensor_scalar(
                            out=v_sb[:, c0 // HD : (c0 + cn) // HD, t, 0:HD],
                            in0=acc[:, 0:cn].rearrange("p (h d) -> p h d", d=HD),
                            scalar1=gcolv[:, t : t + 1], scalar2=None, op0=OP.mult,
                        )
                    nc.vector.tensor_copy(
                        out=v_sb[:, :, t, HD : HD + 1],
                        in_=gcol8[:, t : t + 1].to_broadcast([P, H, 1]),
                    )

            # ---- attention + per-half ffn chain ----
            # The chain for query half 0 (out-proj, LN2, ff1, ff2 -- PE
            # heavy) overlaps with attention for half 1 (exp / ACT
            # heavy): attention mixes tokens only on the key side, so
            # everything after att is independent per query token.
            attT = attp.tile([P, KT, S], bf16, tag="attT")
            h2 = h2pool.tile([P, KT, S], bf16, tag="h2")
            f_sb = fpool.tile([P, FT, S], bf16, tag="f")
            ob_sb = params.tile([P, KT], f32, tag="ob")
            nc.sync.dma_start(out=ob_sb[:, :], in_=d_ob[l])
            f1b_sb = params.tile([P, FT], f32, tag="f1b")
            nc.sync.dma_start(out=f1b_sb[:, :], in_=d_f1b[l])
            f2b_sb = params.tile([P, KT], f32, tag="f2b")
            nc.sync.dma_start(out=f2b_sb[:, :], in_=d_f2b[l])

            for n in range(NQ):
                nsl = ds(n * QW, QW)
                for h in range(H):
                    g, slot = h // 4, h % 4
                    sp = ds(32 * slot, 32)
                    pat = patt.tile([HD + 2, QW], f32, tag="patt")
                    for b in range(NT // 2):
                        e = epool.tile([P, 2, QW], fp8, tag="e")
                        ps = pscore.tile([P, 2, QW], f32, tag="ps")
                        for i in range(2):
                            nc.tensor.matmul(
                                ps[:, i, :],
                                qk_sb[sp, 3 + g, :, ts(2 * b + i, P)],
                                qk_sb[sp, g, :, nsl],
                                start=True, stop=True,
                                perf_mode=DR,
                                tile_position=(32 * slot, 0),
                            )
                        nc.scalar.activation(
                            e[:, :, :], ps[:, :, :], AF.Exp,
                            scale=1.0 / (SQK * SQK),
                        )
                        nc.tensor.matmul(
                            pat[:, :],
                            v_sb[:, h, 2 * b : 2 * b + 2, 0:66],
                            e[:, :, :],
                            start=(b == 0), stop=(b == NT // 2 - 1),
                            perf_mode=DR,
                        )
                    zinv = srows.tile([1, QW], bf16, tag="zinv")
                    with nc.allow_low_precision(reason="softmax denom"):
                        nc.vector.reciprocal(zinv[:, :], pat[HD : HD + 1, :])
                    zb = zpool.tile([HD, QW], bf16, tag="zb")
                    nc.gpsimd.partition_broadcast(
                        zb[:, :], zinv[:, :], channels=HD
                    )
                    nc.vector.tensor_tensor(
                        out=attT[64 * (h % 2) : 64 * (h % 2) + HD, h // 2, nsl],
                        in0=pat[0:HD, :], in1=zb[:, :], op=OP.mult,
                    )

                # ---- output projection (bf16) + residual, half n ----
                for m in range(KT):
                    if m % 3 == 0:
                        wch = wbf.tile([P, 3, KT, P], bf16, tag="wb")
                        nc.sync.dma_start(
                            out=wch[:, :, :, :],
                            in_=d_ow[l, m : m + 3]
                            .rearrange("m (j p) c -> p m j c", p=P),
                        )
                    wt = wch[:, m % 3]
                    acc = pmm.tile([P, QW], f32, tag="pmm")
                    for j in range(KT):
                        nc.tensor.matmul(
                            acc[:, :], wt[:, j, :], attT[:, j, nsl],
                            start=(j == 0), stop=(j == KT - 1),
                        )
                    dx = tmp.tile([P, QW], bf16, tag="tmp")
                    nc.vector.tensor_scalar(
                        out=dx[:, :], in0=acc[:, :],
                        scalar1=ob_sb[:, m : m + 1], scalar2=None, op0=OP.add,
                    )
                    nc.gpsimd.tensor_tensor(
                        out=xT[:, m, nsl], in0=xT[:, m, nsl], in1=dx[:, :],
                        op=OP.add,
                    )

                # ---- LN2 + feed forward (bf16), half n ----
                layer_norm_half(xT, h2, n)
                for m in range(FT):
                    if m % 3 == 0:
                        fch = wbf.tile([P, 3, KT, P], bf16, tag="wb")
                        nc.sync.dma_start(
                            out=fch[:, :, :, :],
                            in_=d_f1w[l, m : m + 3]
                            .rearrange("m (j p) c -> p m j c", p=P),
                        )
                    wt = fch[:, m % 3]
                    acc = pmm.tile([P, QW], f32, tag="pmm")
                    for j in range(KT):
                        nc.tensor.matmul(
                            acc[:, :], wt[:, j, :], h2[:, j, nsl],
                            start=(j == 0), stop=(j == KT - 1),
                        )
                    nc.scalar.activation(
                        f_sb[:, m, nsl], acc[:, :], AF.Relu,
                        bias=f1b_sb[:, m : m + 1],
                    )
                for m in range(KT):
                    acc = pmm.tile([P, QW], f32, tag="pmm")
                    w2 = wff2.tile([P, FT, P], bf16, tag="wff2")
                    nc.sync.dma_start(
                        out=w2[:, :, :],
                        in_=d_f2w[l, m].rearrange("(j p) c -> p j c", p=P),
                    )
                    for jj in range(FT):
                        nc.tensor.matmul(
                            acc[:, :], w2[:, jj, :],
                            f_sb[:, jj, nsl],
                            start=(jj == 0), stop=(jj == FT - 1),
                        )
                    dx = tmp.tile([P, QW], bf16, tag="tmp")
                    nc.vector.tensor_scalar(
                        out=dx[:, :], in0=acc[:, :],
                        scalar1=f2b_sb[:, m : m + 1], scalar2=None, op0=OP.add,
                    )
                    nc.gpsimd.tensor_tensor(
                        out=xT[:, m, nsl], in0=xT[:, m, nsl], in1=dx[:, :],
                        op=OP.add,
                    )

        # ---------------- CLS head (f32, token cols 0:2) ----------------
        col2 = xT[:, :, 0:2]
        xsqc = consts.tile([P, KT, 2], f32r, tag="xsqc")
        nc.vector.tensor_tensor(out=xsqc[:, :, :], in0=col2, in1=col2, op=OP.mult)
        pss = pmm.tile([1, QW], f32, tag="pmm")
        for j in range(KT):
            nc.tensor.matmul(
                pss[:, 0:2], ones_col[:, :], xT[:, j, 0:2],
                start=(j == 0), stop=(j == KT - 1),
            )
        for j in range(KT):
            nc.tensor.matmul(
                pss[:, 2:4], ones_col[:, :], xsqc[:, j, :],
                start=(j == 0), stop=(j == KT - 1),
            )
        hmean = srows.tile([1, 8], f32r, tag="hmean")
        nc.vector.tensor_scalar(
            out=hmean[:, 0:2], in0=pss[:, 0:2], scalar1=1.0 / D, scalar2=None,
            op0=OP.mult,
        )
        hmsq = srows.tile([1, 8], f32, tag="hsrow")
        nc.vector.tensor_scalar(
            out=hmsq[:, 0:2], in0=pss[:, 2:4], scalar1=1.0 / D, scalar2=None,
            op0=OP.mult,
        )
        hvar = srows.tile([1, 8], f32, tag="hsrow")
        nc.vector.scalar_tensor_tensor(
            out=hvar[:, 0:2], in0=hmean[:, 0:2], scalar=-1.0, in1=hmean[:, 0:2],
            op0=OP.mult, op1=OP.mult,
        )
        nc.vector.tensor_tensor(
            out=hvar[:, 0:2], in0=hvar[:, 0:2], in1=hmsq[:, 0:2], op=OP.add
        )
        hlnv = srows.tile([1, 8], f32, tag="hsrow")
        nc.scalar.activation(hlnv[:, 0:2], hvar[:, 0:2], AF.Ln, bias=eps_sb[:, :])
        hrstd = srows.tile([1, 8], f32r, tag="hrstd")
        nc.scalar.activation(hrstd[:, 0:2], hlnv[:, 0:2], AF.Exp, scale=-0.5)
        pbc = pmm.tile([P, QW], f32, tag="pmm")
        nc.tensor.matmul(pbc[:, 0:2], ones_rowr[0:1, 0:P], hmean[:, 0:2],
                         start=True, stop=True)
        nc.tensor.matmul(pbc[:, 2:4], ones_rowr[0:1, 0:P], hrstd[:, 0:2],
                         start=True, stop=True)
        t1 = consts.tile([P, KT, 2], f32, tag="ht1")
        nc.vector.tensor_tensor(
            out=t1[:, :, :], in0=col2, in1=pbc[:, 0:1].to_broadcast([P, KT, 2]),
            op=OP.subtract,
        )
        t2 = consts.tile([P, KT, 2], f32, tag="ht2")
        nc.vector.tensor_tensor(
            out=t2[:, :, :], in0=t1[:, :, :], in1=pbc[:, 2:3].to_broadcast([P, KT, 2]),
            op=OP.mult,
        )
        t3 = consts.tile([P, KT, 2], f32, tag="ht3")
        nc.vector.tensor_tensor(
            out=t3[:, :, :], in0=t2[:, :, :], in1=hls_sb[:, :].to_broadcast([P, KT, 2]),
            op=OP.mult,
        )
        pc = consts.tile([P, KT, 2], f32r, tag="pc")
        nc.vector.tensor_tensor(
            out=pc[:, :, :], in0=t3[:, :, :], in1=hlb_sb[:, :].to_broadcast([P, KT, 2]),
            op=OP.add,
        )
        plog = patt.tile([HD + 1, QW], f32, tag="patt")
        nc.tensor.matmul(
            plog[0:C, 0:2], cb_sb[:, :], ones_rowr[:, 0:2], start=True, stop=False
        )
        for j in range(KT):
            nc.tensor.matmul(
                plog[0:C, 0:2], cw_sb[:, j, :], pc[:, j, :],
                start=False, stop=(j == KT - 1),
            )
        out_sb = consts.tile([C, 1], f32, tag="outsb")
        nc.vector.tensor_copy(out=out_sb[:, :], in_=plog[0:C, 0:1])
        nc.sync.dma_start(out=d_out[0:1, 0:C], in_=out_sb[0:C, 0:1])

    nc.compile()
    return nc


def _bf16np():
    import ml_dtypes

    return ml_dtypes.bfloat16


def _f8np():
    import ml_dtypes

    return ml_dtypes.float8_e4m3


def _q8(x):
    return np.clip(x, -240.0, 240.0).astype(_f8np())


def _prep_host(inputs):
    g = lambda k: np.asarray(inputs[k])
    sq = np.float32(math.sqrt(D))
    ids = g("input_ids").astype(np.int32)              # (B, S)
    gm = (1.0 - g("attention_mask").astype(np.float32))  # (B, S)
    emb = (g("token_emb").astype(np.float32) * sq)
    posT = np.ascontiguousarray((g("pos_emb")[:S].astype(np.float32) * sq).T).astype(_bf16np())

    # reference reshapes qkv output to (H, 3, HD): permute columns into
    # contiguous q | k | v blocks (each h-major)
    idx = np.arange(3 * D).reshape(H, 3, HD)
    cols = np.concatenate(
        [idx[:, 0, :].reshape(-1), idx[:, 1, :].reshape(-1), idx[:, 2, :].reshape(-1)]
    )
    qkv_w0 = g("qkv_w").astype(np.float32)[:, :, cols]        # (L, D, 3D)
    qkv_b0 = g("qkv_b").astype(np.float32)[:, cols]           # (L, 3D)
    n1s, n1b = g("n1_s").astype(np.float32), g("n1_b").astype(np.float32)
    n2s, n2b = g("n2_s").astype(np.float32), g("n2_b").astype(np.float32)

    # fold LN1 scale/bias into qkv:  ((x-m)r) @ (diag(s) W) + (b + n1_b @ W)
    qkv_w = n1s[:, :, None] * qkv_w0
    qkv_b = qkv_b0 + np.einsum("ld,lde->le", n1b, qkv_w0)

    # the score scale 1/sqrt(HD) = 1/8 is split evenly across q and k
    r8 = np.float32(math.sqrt(1.0 / math.sqrt(HD)))           # 1/sqrt(8)

    qw = qkv_w[:, :, :D] * (r8 * SQK)
    qb = qkv_b[:, :D] * (r8 * SQK)
    kw = qkv_w[:, :, D : 2 * D] * (r8 * SQK)
    kb = qkv_b[:, D : 2 * D] * (r8 * SQK)
    vw = qkv_w[:, :, 2 * D :] * SV
    vb = qkv_b[:, 2 * D :] * SV

    # qk tile layout: tile t=2g+sub (q), 6+2g+sub (k); within a tile the
    # 128 columns are 32*slot + (d%32) for heads h=4g+slot, dims with
    # d//32 == sub.
    def qk_tiles(w):                                          # (L, D, 768)
        w = w.reshape(L, D, 3, 4, 2, 32)                      # g slot sub r
        w = w.transpose(0, 2, 4, 1, 3, 5)                     # L g sub D slot r
        return w.reshape(L, 6, D, P)

    def qk_btiles(b):                                         # (L, 768)
        b = b.reshape(L, 3, 4, 2, 32).transpose(0, 1, 3, 2, 4)
        return b.reshape(L, 6, P)

    qkw_t = np.concatenate([qk_tiles(qw), qk_tiles(kw)], axis=1)    # (L,12,D,P)
    qkb_t = np.concatenate([qk_btiles(qb), qk_btiles(kb)], axis=1)  # (L,12,P)

    ow0 = g("out_w").astype(np.float32)                       # (L, D, D)
    ob0 = g("out_b").astype(np.float32)

    ff1_w0 = g("ff1_w").astype(np.float32)                    # (L, D, DF)
    ff1_w = n2s[:, :, None] * ff1_w0
    ff1_b = g("ff1_b").astype(np.float32) + np.einsum("ld,lde->le", n2b, ff1_w0)
    ff2_w0 = g("ff2_w").astype(np.float32)                    # (L, DF, D)
    ff2_b0 = g("ff2_b").astype(np.float32)

    shared = {
        "emb": emb,
        "posT": posT,
        "qkw": _q8(np.ascontiguousarray(qkw_t)),
        "qkb": np.ascontiguousarray(qkb_t.transpose(0, 2, 1)),      # (L,P,12)
        "vw": _q8(np.ascontiguousarray(vw.reshape(L, KT, P, D))),
        "vb": _q8(vb),
        "ow": np.ascontiguousarray(
            ow0.reshape(L, D, KT, P).transpose(0, 2, 1, 3)).astype(_bf16np()),
        "ob": np.ascontiguousarray(ob0.reshape(L, KT, P).transpose(0, 2, 1)),
        "f1w": np.ascontiguousarray(
            ff1_w.reshape(L, D, FT, P).transpose(0, 2, 1, 3)).astype(_bf16np()),
        "f1b": np.ascontiguousarray(ff1_b.reshape(L, FT, P).transpose(0, 2, 1)),
        "f2w": np.ascontiguousarray(
            ff2_w0.reshape(L, DF, KT, P).transpose(0, 2, 1, 3)).astype(_bf16np()),
        "f2b": np.ascontiguousarray(ff2_b0.reshape(L, KT, P).transpose(0, 2, 1)),
        "hls": np.ascontiguousarray(g("hln_s").astype(np.float32).reshape(KT, P).T),
        "hlb": np.ascontiguousarray(g("hln_b").astype(np.float32).reshape(KT, P).T),
        "cw": g("cls_w").astype(np.float32),
        "cb": g("cls_b").astype(np.float32).reshape(1, C),
    }
    per_core = []
    for c in range(N_CORES):
        per_core.append(
            {
                "ids": np.ascontiguousarray(ids[c].reshape(NT, P).T),
                "gmask": np.ascontiguousarray(gm[c].reshape(NT, P).T),
            }
        )
    return shared, per_core


def _get_nc():
    if "nc" not in _CACHE:
        _CACHE["nc"] = _build_nc()
    return _CACHE["nc"]


def kernel(**inputs):
    from concourse.bass_utils import run_bass_kernel_spmd

    shared, per_core = _prep_host(inputs)
    nc = _get_nc()
    in_maps = [dict(shared, **per_core[c]) for c in range(N_CORES)]
    _CACHE["in_maps"] = in_maps
    res = run_bass_kernel_spmd(nc, in_maps, list(range(N_CORES)))
    out = np.stack([res.results[c]["out"][0] for c in range(N_CORES)], axis=0)
    return out.astype(np.float32)


def bench(n_iters=10):
    """Re-run the compiled NEFF with device-resident inputs; returns the
    best-observed per-iteration wall time in ns (upper bound on HW exec)."""
    import time

    import jax
    import numpy as _np
    from jax.sharding import Mesh, PartitionSpec, NamedSharding
    from jax.experimental.shard_map import shard_map
    from concourse import bass2jax, mybir
    from concourse.bass2jax import _bass_exec_p, install_neuronx_cc_hook

    nc = _get_nc()
    in_maps = _CACHE["in_maps"]
    install_neuronx_cc_hook()

    pname = nc.partition_id_tensor.name if nc.partition_id_tensor else None
    in_names, out_names, out_avals, zero_outs = [], [], [], []
    for alloc in nc.m.functions[0].allocations:
        if not isinstance(alloc, mybir.MemoryLocationSet):
            continue
        name = alloc.memorylocations[0].name
        if alloc.kind == "ExternalInput":
            if name == pname:
                continue
            in_names.append(name)
        elif alloc.kind == "ExternalOutput":
            out_names.append(name)
            shape = tuple(alloc.tensor_shape)
            dtype = mybir.dt.np(alloc.dtype)
            out_avals.append(jax.core.ShapedArray(shape, dtype))
            zero_outs.append(_np.zeros(shape, dtype))
    n_params = len(in_names)
    all_names = in_names + out_names
    if pname is not None:
        all_names = all_names + [pname]

    def _body(*args):
        operands = list(args)
        if pname is not None:
            operands.append(bass2jax.partition_id_tensor())
        outs = _bass_exec_p.bind(
            *operands,
            out_avals=tuple(out_avals),
            in_names=tuple(all_names),
            out_names=tuple(out_names),
            lowering_input_output_aliases=(),
            sim_require_finite=True,
            sim_require_nnan=True,
            nc=nc,
        )
        return tuple(outs)

    devices = jax.devices()[:N_CORES]
    mesh = Mesh(_np.asarray(devices), ("core",))
    nin = n_params + len(zero_outs)
    fn = jax.jit(
        shard_map(
            _body,
            mesh=mesh,
            in_specs=(PartitionSpec("core"),) * nin,
            out_specs=(PartitionSpec("core"),) * len(out_names),
            check_rep=False,
        )
    )
    sharding = NamedSharding(mesh, PartitionSpec("core"))
    concat_in = [
        jax.device_put(
            _np.concatenate([_np.asarray(in_maps[c][n]) for c in range(N_CORES)], 0),
            sharding,
        )
        for n in in_names
    ]
    concat_zeros = [
        jax.device_put(
            _np.zeros((N_CORES * z.shape[0], *z.shape[1:]), z.dtype), sharding
        )
        for z in zero_outs
    ]
    jax.block_until_ready(concat_in)
    # warmup (compile)
    out = fn(*concat_in, *concat_zeros)
    jax.block_until_ready(out)
    # pipelined async dispatch amortizes the axon tunnel round-trip
    outs = []
    t0 = time.perf_counter()
    for _ in range(n_iters):
        outs.append(fn(*concat_in, *concat_zeros))
    jax.block_until_ready(outs)
    dt = (time.perf_counter() - t0) / n_iters
    return int(dt * 1e9)
